# revision 23
# baseline (speedup 1.0000x reference)
"""Trainium2 Bass kernel for nn_Attention_79748952752529.

Masked softmax attention with post-softmax additive bias (beta), QKV
projections fused. Batch-sharded across 8 NeuronCores (1 batch element per
core); beta is replicated (streamed) to every core, pre-transposed on host.

Math per core (batch b), all on-chip tensors transposed (d on partitions):
  qpT = (q W_q^T + b_q)^T          [DIM, TQ]
  kpT = (k W_k^T + b_k)^T          [DIM, TK]
  vp  = v W_v^T + b_v              [TK, DIM]  (head-interleaved; two copies:
                                    plain, and src-masked with ones column)
  per head h:
    S.T  = kpT_h^T qpT_h           [TK, TQ]   (K=64, two heads packed in PE)
    E    = exp(S.T/32)                        (unmasked exp; mask is folded
                                               into the masked-vp lhsT below)
    O_E  = [m*v_h | m]^T E         [65, TQ]   (m = src mask; row 64 = masked
                                               softmax denominator)
    O_B  = v_h^T betaT_h           [64, TQ]
    outT_h = O_E[0:64] * (tgt/denom)[tq] + O_B
Host fixes rows where tgt_mask=0 (softmax of all-masked row is uniform
1/TK): out[b, tq, :] += (sum_t v[b] @ W_v^T + TK*b_v) / TK.
"""

import sys

for _p in ("/opt/trn_rl_repo",):
    if _p in sys.path:
        sys.path.remove(_p)

from contextlib import ExitStack

import ml_dtypes
import numpy as np

import concourse.bacc as bacc
import concourse.bass as bass
import concourse.mybir as mybir
import concourse.tile as tile

BF16 = mybir.dt.bfloat16
F32 = mybir.dt.float32
NPBF16 = ml_dtypes.bfloat16

# Full problem config
B, TQ, TK, DIM, H = 8, 1024, 1024, 1024, 16
D = DIM // H
P = 128
N_CORES = 8


class Cfg:
    def __init__(self, tq=TQ, tk=TK, dim=DIM, h=H):
        self.tq, self.tk, self.dim, self.h = tq, tk, dim, h
        self.d = dim // h
        assert self.d == 64, "kernel assumes head dim 64 (2 heads per 128 partitions)"
        self.nt_q = tq // P          # tq partition tiles
        self.nt_k = tk // P          # tk partition tiles
        self.nt_d = dim // P         # dim partition tiles (also: head pairs)
        self.tqb = min(512, tq)      # tq free-dim block (one PSUM bank of fp32)
        self.n_tqb = tq // self.tqb
        self.scale = float(dim) ** -0.5


def build_kernel(cfg: Cfg):
    """Build and compile the per-core Bass program. Returns nc."""
    nc = bacc.Bacc("TRN2", target_bir_lowering=False, debug=False)

    qT = nc.dram_tensor("qT", [cfg.dim, cfg.tq], BF16, kind="ExternalInput").ap()
    kT = nc.dram_tensor("kT", [cfg.dim, cfg.tk], BF16, kind="ExternalInput").ap()
    vT = nc.dram_tensor("vT", [cfg.dim, cfg.tk], BF16, kind="ExternalInput").ap()
    WqT = nc.dram_tensor("WqT", [cfg.dim, cfg.dim], BF16, kind="ExternalInput").ap()
    WkT = nc.dram_tensor("WkT", [cfg.dim, cfg.dim], BF16, kind="ExternalInput").ap()
    WvT = nc.dram_tensor("WvT", [cfg.dim, cfg.dim], BF16, kind="ExternalInput").ap()
    bqT = nc.dram_tensor("bqT", [P, cfg.nt_d], F32, kind="ExternalInput").ap()
    bkT = nc.dram_tensor("bkT", [P, cfg.nt_d], F32, kind="ExternalInput").ap()
    bv_rep = nc.dram_tensor("bv_rep", [P, cfg.dim], F32, kind="ExternalInput").ap()
    srcT_f = nc.dram_tensor("srcT_f", [P, cfg.nt_k], F32, kind="ExternalInput").ap()
    srcT_b = nc.dram_tensor("srcT_b", [P, cfg.nt_k], BF16, kind="ExternalInput").ap()
    tgt_row = nc.dram_tensor("tgt_row", [1, cfg.tq], F32, kind="ExternalInput").ap()
    betaT = nc.dram_tensor(
        "betaT", [cfg.h, cfg.tk, cfg.tq], BF16, kind="ExternalInput"
    ).ap()
    outT = nc.dram_tensor("outT", [cfg.dim, cfg.tq], F32, kind="ExternalOutput").ap()

    with tile.TileContext(nc) as tc, ExitStack() as ctx:
        consts = ctx.enter_context(tc.tile_pool(name="consts", bufs=1))
        proj_out = ctx.enter_context(tc.tile_pool(name="projout", bufs=1))
        dram_s = ctx.enter_context(tc.tile_pool(name="dram_s", bufs=2, space="DRAM"))
        ps_sc = ctx.enter_context(tc.tile_pool(name="ps_sc", bufs=2, space="PSUM"))
        ps_pv = ctx.enter_context(tc.tile_pool(name="ps_pv", bufs=1, space="PSUM"))
        ps_pb = ctx.enter_context(tc.tile_pool(name="ps_pb", bufs=1, space="PSUM"))

        # qpT/kpT: [p, ot, t] bf16
        # vp_m: src-masked, [p, tt, h, 65] (col 64 = src mask); vp_p: plain
        qp_sb = proj_out.tile([P, cfg.nt_d, cfg.tq], BF16, tag="qp")
        kp_sb = proj_out.tile([P, cfg.nt_d, cfg.tk], BF16, tag="kp")
        vp_m = proj_out.tile([P, cfg.nt_k, cfg.h, D + 1], BF16, tag="vpm")
        vp_p = proj_out.tile([P, cfg.nt_k, cfg.h, D], BF16, tag="vpp")

        # ---- projections (weight/input pools scoped: SBUF recycles after) ----
        with tc.tile_pool(name="wpool", bufs=1) as w_pool, tc.tile_pool(
            name="inp", bufs=2
        ) as in_pool:
            w_sb = {}
            CH = max(1, cfg.nt_d // 4)  # dtiles per DMA chunk

            def load_w(name, w, interleave=None):
                t = w_pool.tile(
                    [P, cfg.nt_d, cfg.dim], BF16, tag=f"w_{name}", name=f"w_{name}"
                )
                wr = w.rearrange("(dt p) o -> p dt o", p=P)
                for c in range(cfg.nt_d // CH):
                    cs = slice(c * CH, (c + 1) * CH)
                    nc.sync.dma_start(t[:, cs, :], wr[:, cs, :])
                    if interleave is not None:
                        xtile, xr, tqs = interleave
                        nc.sync.dma_start(xtile[:, cs, :], xr[:, cs, tqs])
                w_sb[name] = t

            # first projection's input is interleaved with its weight chunks
            # so the first matmul group isn't stuck behind the whole queue
            x0 = in_pool.tile([P, cfg.nt_d, cfg.tqb], BF16, tag="xin", name="x0")
            qr0 = qT.rearrange("(dt p) t -> p dt t", p=P)
            load_w("wq", WqT, interleave=(x0, qr0, slice(0, cfg.tqb)))
            # small resident constants (emitted after the first weight chunks
            # so they don't delay the first matmul)
            bq_sb = consts.tile([P, cfg.nt_d], F32, tag="bq")
            nc.sync.dma_start(bq_sb[:], bqT)
            bk_sb = consts.tile([P, cfg.nt_d], F32, tag="bk")
            nc.sync.dma_start(bk_sb[:], bkT)
            bv_sb = consts.tile([P, cfg.dim], F32, tag="bv")
            nc.sync.dma_start(bv_sb[:], bv_rep)
            src_sb = consts.tile([P, cfg.nt_k], F32, tag="src")
            nc.sync.dma_start(src_sb[:], srcT_f)
            srcb_sb = consts.tile([P, cfg.nt_k], BF16, tag="srcb")
            nc.sync.dma_start(srcb_sb[:], srcT_b)
            # tgt row lives at partition 64 (same as the PV denominator row)
            tgt_sb = consts.tile([P, cfg.tq], F32, tag="tgt")
            nc.sync.dma_start(tgt_sb[64:65, :], tgt_row)

            # qpT / kpT  (out tile [o=128, t=tqb])
            for name, wdram, src, dst, bias in (
                ("wq", None, qT, qp_sb, bq_sb),
                ("wk", WkT, kT, kp_sb, bk_sb),
            ):
                if wdram is not None:
                    load_w(name, wdram)
                w = w_sb[name]
                for tb in range(cfg.n_tqb):
                    if name == "wq" and tb == 0:
                        x = x0
                    else:
                        x = in_pool.tile([P, cfg.nt_d, cfg.tqb], BF16, tag="xin")
                        xr = src.rearrange("(dt p) t -> p dt t", p=P)
                        for c in range(cfg.nt_d // CH):
                            cs = slice(c * CH, (c + 1) * CH)
                            nc.sync.dma_start(
                                x[:, cs, :],
                                xr[:, cs, tb * cfg.tqb : (tb + 1) * cfg.tqb],
                            )
                    for ot in range(cfg.nt_d):
                        ps = ps_sc.tile([P, 2, cfg.tqb], F32, tag="ps")
                        for dt in range(cfg.nt_d):
                            nc.tensor.matmul(
                                ps[:, 0, :],
                                w[:, dt, ot * P : (ot + 1) * P],
                                x[:, dt, :],
                                start=(dt == 0),
                                stop=(dt == cfg.nt_d - 1),
                            )
                        nc.vector.tensor_add(
                            dst[:, ot, tb * cfg.tqb : (tb + 1) * cfg.tqb],
                            ps[:, 0, :],
                            bias[:, ot : ot + 1].to_broadcast([P, cfg.tqb]),
                        )

            # vp (out tile [t=128, o=OB]); plain + masked copies
            OB = min(512, cfg.dim)
            n_ob = cfg.dim // OB
            hpb = OB // D  # heads per block
            load_w("wv", WvT)
            wv = w_sb["wv"]
            # v input loaded once (8 chunks), sliced per t-tile below
            xv = in_pool.tile([P, cfg.nt_d, cfg.tk], BF16, tag="xv")
            xvr = vT.rearrange("(dt p) t -> p dt t", p=P)
            for c in range(cfg.nt_d // CH):
                cs = slice(c * CH, (c + 1) * CH)
                nc.sync.dma_start(xv[:, cs, :], xvr[:, cs, :])
            for tt in range(cfg.nt_k):
                x = xv[:, :, tt * P : (tt + 1) * P]
                for ob in range(n_ob):
                    ps = ps_sc.tile([P, 2, cfg.tqb], F32, tag="ps")
                    for dt in range(cfg.nt_d):
                        nc.tensor.matmul(
                            ps[:, 0, :OB],
                            x[:, dt, :],
                            wv[:, dt, ob * OB : (ob + 1) * OB],
                            start=(dt == 0),
                            stop=(dt == cfg.nt_d - 1),
                        )
                    hsl = slice(ob * hpb, (ob + 1) * hpb)
                    nc.vector.tensor_add(
                        vp_p[:, tt, hsl, :],
                        ps[:, 0, :OB].rearrange("p (h d) -> p h d", d=D),
                        bv_sb[:, ob * OB : (ob + 1) * OB].rearrange(
                            "p (h d) -> p h d", d=D
                        ),
                    )
                    nc.vector.tensor_scalar_mul(
                        vp_m[:, tt, hsl, 0:D],
                        vp_p[:, tt, hsl, :],
                        src_sb[:, tt : tt + 1],
                    )

        # src-mask ones column of vp_m (DVE free-dim broadcast, not DMA)
        nc.vector.tensor_copy(
            vp_m[:, :, :, D],
            srcb_sb[:, :, None].to_broadcast([P, cfg.nt_k, cfg.h]),
        )

        # ---- attention: software-pipelined over head pairs ----
        # Per pair: scores+exp run one pair AHEAD of the PV/fixup stage, and
        # their PE instructions are interleaved per k-tile group so the
        # in-order PE queue never waits on the exp (ACT) pipeline.
        e_pool = ctx.enter_context(tc.tile_pool(name="epool", bufs=2))
        b_pool = ctx.enter_context(tc.tile_pool(name="bpool", bufs=4))
        s_pool = ctx.enter_context(tc.tile_pool(name="spool", bufs=2))
        o_pool = ctx.enter_context(tc.tile_pool(name="opool", bufs=3))
        pairs = [
            (tb, j) for tb in range(cfg.n_tqb) for j in range(cfg.h // 2)
        ]
        prev = None  # state of pair idx-1: dict with e_t, bsl, ps_e, ps_b, tqs, j

        def emit_scores_exp(state, kt2):
            """Two heads packed in PE rows 0-63/64-127; one exp per 2 k-tiles."""
            j, tqs = state["j"], state["tqs"]
            for half in range(2):
                r0 = half * 64
                ps = ps_sc.tile([P, 2, cfg.tqb], F32, tag="ps", name="ps")
                for ki in range(2):
                    kt = 2 * kt2 + ki
                    nc.tensor.matmul(
                        ps[:, ki, :],
                        kp_sb[r0 : r0 + 64, j, kt * P : (kt + 1) * P],
                        qp_sb[r0 : r0 + 64, j, tqs],
                        start=True,
                        stop=True,
                    )
                nc.scalar.activation(
                    state["e_t"][half][:, 2 * kt2 : 2 * kt2 + 2, :],
                    ps[:],
                    mybir.ActivationFunctionType.Exp,
                    scale=cfg.scale,
                )

        def emit_pv(state, kt):
            st, sp = kt == 0, kt == cfg.nt_k - 1
            if st:
                state["ps_e"] = [
                    ps_pv.tile([P, cfg.tqb], F32, tag=f"pse{h}", name=f"pse{h}")
                    for h in range(2)
                ]
                state["ps_b"] = [
                    ps_pb.tile([P, cfg.tqb], F32, tag=f"psb{h}", name=f"psb{h}")
                    for h in range(2)
                ]
            for half in range(2):
                hh = 2 * state["j"] + half
                nc.tensor.matmul(
                    state["ps_e"][half][0 : D + 1, :],
                    vp_m[:, kt, hh, :],
                    state["e_t"][half][:, kt, :],
                    start=st,
                    stop=sp,
                )
                nc.tensor.matmul(
                    state["ps_b"][half][0:D, :],
                    vp_p[:, kt, hh, :],
                    state["bsl"][half][:, kt, :],
                    start=st,
                    stop=sp,
                )

        def emit_fixup(state):
            j, tqs = state["j"], state["tqs"]
            for half in range(2):
                hh = 2 * j + half
                # Drain both PSUM accumulators to SBUF right away (ACT + DVE)
                # so the banks free for the next pair's PV groups; the slow
                # normalization chain below then runs entirely from SBUF.
                oe = o_pool.tile([D + 1, cfg.tqb], F32, tag="oe", name="oe")
                nc.scalar.activation(
                    oe[:],
                    state["ps_e"][half][0 : D + 1, :],
                    mybir.ActivationFunctionType.Copy,
                )
                ob = o_pool.tile([D, cfg.tqb], F32, tag="ob", name="ob")
                nc.vector.tensor_copy(ob[:], state["ps_b"][half][0:D, :])
                # s = tgt / denom  (denominator row sits at partition 64)
                srow = s_pool.tile([P, cfg.tqb], F32, tag="srow", name="srow")
                nc.vector.reciprocal(srow[64:65, :], oe[64:65, :])
                nc.vector.tensor_mul(
                    srow[64:65, :], srow[64:65, :], tgt_sb[64:65, tqs]
                )
                s_dram = dram_s.tile([1, cfg.tqb], F32, tag="sdram", name="sdram")
                nc.sync.dma_start(s_dram[:], srow[64:65, :])
                s_rep = s_pool.tile([64, cfg.tqb], F32, tag="srep", name="srep")
                nc.sync.dma_start(s_rep[:], s_dram[:].to_broadcast([64, cfg.tqb]))
                tmp = o_pool.tile([64, cfg.tqb], F32, tag="tmp", name="tmp")
                nc.vector.tensor_mul(tmp[:], oe[0:D, :], s_rep[:])
                osb = o_pool.tile([64, cfg.tqb], F32, tag="osb", name="osb")
                nc.vector.tensor_add(osb[:], tmp[:], ob[:])
                nc.sync.dma_start(outT[hh * D : (hh + 1) * D, tqs], osb[:])

        for idx, (tb, j) in enumerate(pairs):
            tqs = slice(tb * cfg.tqb, (tb + 1) * cfg.tqb)
            state = {"j": j, "tqs": tqs}
            state["e_t"] = [
                e_pool.tile(
                    [P, cfg.nt_k, cfg.tqb], BF16, tag=f"e{h}", name=f"e{h}"
                )
                for h in range(2)
            ]
            state["bsl"] = []
            for half in range(2):
                hh = 2 * j + half
                bt = b_pool.tile(
                    [P, cfg.nt_k, cfg.tqb], BF16, tag="beta", name=f"beta{half}"
                )
                nc.sync.dma_start(
                    bt[:],
                    betaT[hh].rearrange("(kt p) t -> p kt t", p=P)[:, :, tqs],
                )
                state["bsl"].append(bt)
            for kt2 in range(cfg.nt_k // 2):
                emit_scores_exp(state, kt2)
                if prev is not None:
                    emit_pv(prev, 2 * kt2)
                    emit_pv(prev, 2 * kt2 + 1)
            if prev is not None:
                emit_fixup(prev)
            prev = state
        for kt in range(cfg.nt_k):
            emit_pv(prev, kt)
        emit_fixup(prev)

    nc.compile()
    return nc


def host_prep(cfg: Cfg, q, k, v, beta, src_mask, tgt_mask, Wq, bq, Wk, bk, Wv, bv):
    """Build per-core input maps (host-side sharding + transposition)."""
    WqT = np.ascontiguousarray(Wq.T).astype(NPBF16)
    WkT = np.ascontiguousarray(Wk.T).astype(NPBF16)
    WvT = np.ascontiguousarray(Wv.T).astype(NPBF16)
    bqT = np.ascontiguousarray(bq.reshape(cfg.nt_d, P).T).astype(np.float32)
    bkT = np.ascontiguousarray(bk.reshape(cfg.nt_d, P).T).astype(np.float32)
    bv_rep = np.ascontiguousarray(np.broadcast_to(bv, (P, cfg.dim))).astype(np.float32)
    betaT = np.ascontiguousarray(beta.transpose(0, 2, 1)).astype(NPBF16)

    in_maps = []
    for b in range(q.shape[0]):
        srcT = np.ascontiguousarray(
            src_mask[b].astype(np.float32).reshape(cfg.nt_k, P).T
        )
        in_maps.append(
            {
                "qT": np.ascontiguousarray(q[b].T).astype(NPBF16),
                "kT": np.ascontiguousarray(k[b].T).astype(NPBF16),
                "vT": np.ascontiguousarray(v[b].T).astype(NPBF16),
                "WqT": WqT,
                "WkT": WkT,
                "WvT": WvT,
                "bqT": bqT,
                "bkT": bkT,
                "bv_rep": bv_rep,
                "srcT_f": srcT,
                "srcT_b": srcT.astype(NPBF16),
                "tgt_row": tgt_mask[b].astype(np.float32).reshape(1, cfg.tq),
                "betaT": betaT,
            }
        )
    return in_maps


def host_finish(cfg: Cfg, results, v, tgt_mask, Wv, bv):
    """Assemble full output; patch uniform-softmax rows where tgt_mask=0."""
    nb = v.shape[0]
    out = np.empty((nb, cfg.tq, cfg.dim), np.float32)
    for b in range(nb):
        out[b] = results[b]["outT"].T
        inv = ~tgt_mask[b]
        if inv.any():
            vsum = v[b].sum(axis=0, dtype=np.float64) @ Wv.T.astype(
                np.float64
            ) + cfg.tk * bv.astype(np.float64)
            out[b, inv, :] += (vsum / cfg.tk).astype(np.float32)
    return out


_NC = None


def kernel(q, k, v, beta, src_mask, tgt_mask, Wq, bq, Wk, bk, Wv, bv):
    global _NC
    from concourse.bass_utils import run_bass_kernel_spmd

    q = np.asarray(q, np.float32)
    k = np.asarray(k, np.float32)
    v = np.asarray(v, np.float32)
    beta = np.asarray(beta, np.float32)
    src_mask = np.asarray(src_mask, bool)
    tgt_mask = np.asarray(tgt_mask, bool)
    Wq, bq = np.asarray(Wq, np.float32), np.asarray(bq, np.float32)
    Wk, bk = np.asarray(Wk, np.float32), np.asarray(bk, np.float32)
    Wv, bv = np.asarray(Wv, np.float32), np.asarray(bv, np.float32)

    cfg = Cfg()
    if _NC is None:
        _NC = build_kernel(cfg)
    in_maps = host_prep(cfg, q, k, v, beta, src_mask, tgt_mask, Wq, bq, Wk, bk, Wv, bv)
    res = run_bass_kernel_spmd(_NC, in_maps, list(range(N_CORES)))
    return host_finish(cfg, res.results, v, tgt_mask, Wv, bv)


# revision 28
# speedup vs baseline: 1.2392x; 1.2392x over previous
"""Trainium2 Bass kernel for nn_Attention_79748952752529.

Masked softmax attention with post-softmax additive bias (beta), QKV
projections fused. Batch-sharded across 8 NeuronCores (1 batch element per
core); beta is replicated (streamed) to every core, pre-transposed on host.

Math per core (batch b), all on-chip tensors transposed (d on partitions):
  qpT = (q W_q^T + b_q)^T          [DIM, TQ]
  kpT = (k W_k^T + b_k)^T          [DIM, TK]
  vp  = v W_v^T + b_v              [TK, DIM]  (head-interleaved; two copies:
                                    plain, and src-masked with ones column)
  per head h:
    S.T  = kpT_h^T qpT_h           [TK, TQ]   (K=64, two heads packed in PE)
    E    = exp(S.T/32)                        (unmasked exp; mask is folded
                                               into the masked-vp lhsT below)
    O_E  = [m*v_h | m]^T E         [65, TQ]   (m = src mask; row 64 = masked
                                               softmax denominator)
    O_B  = v_h^T betaT_h           [64, TQ]
    outT_h = O_E[0:64] * (tgt/denom)[tq] + O_B
Host fixes rows where tgt_mask=0 (softmax of all-masked row is uniform
1/TK): out[b, tq, :] += (sum_t v[b] @ W_v^T + TK*b_v) / TK.
"""

import sys

for _p in ("/opt/trn_rl_repo",):
    if _p in sys.path:
        sys.path.remove(_p)

from contextlib import ExitStack

import ml_dtypes
import numpy as np

import concourse.bacc as bacc
import concourse.bass as bass
import concourse.mybir as mybir
import concourse.tile as tile

BF16 = mybir.dt.bfloat16
F32 = mybir.dt.float32
NPBF16 = ml_dtypes.bfloat16

# Full problem config
B, TQ, TK, DIM, H = 8, 1024, 1024, 1024, 16
D = DIM // H
P = 128
N_CORES = 8


class Cfg:
    def __init__(self, tq=TQ, tk=TK, dim=DIM, h=H):
        self.tq, self.tk, self.dim, self.h = tq, tk, dim, h
        self.d = dim // h
        assert self.d == 64, "kernel assumes head dim 64 (2 heads per 128 partitions)"
        self.nt_q = tq // P          # tq partition tiles
        self.nt_k = tk // P          # tk partition tiles
        self.nt_d = dim // P         # dim partition tiles (also: head pairs)
        self.tqb = min(512, tq)      # tq free-dim block (one PSUM bank of fp32)
        self.n_tqb = tq // self.tqb
        self.scale = float(dim) ** -0.5


def build_kernel(cfg: Cfg):
    """Build and compile the per-core Bass program. Returns nc."""
    nc = bacc.Bacc("TRN2", target_bir_lowering=False, debug=False)

    qT = nc.dram_tensor("qT", [cfg.dim, cfg.tq], BF16, kind="ExternalInput").ap()
    kT = nc.dram_tensor("kT", [cfg.dim, cfg.tk], BF16, kind="ExternalInput").ap()
    vT = nc.dram_tensor("vT", [cfg.dim, cfg.tk], BF16, kind="ExternalInput").ap()
    WqT = nc.dram_tensor("WqT", [cfg.dim, cfg.dim], BF16, kind="ExternalInput").ap()
    WkT = nc.dram_tensor("WkT", [cfg.dim, cfg.dim], BF16, kind="ExternalInput").ap()
    WvT = nc.dram_tensor("WvT", [cfg.dim, cfg.dim], BF16, kind="ExternalInput").ap()
    bqT = nc.dram_tensor("bqT", [P, cfg.nt_d], F32, kind="ExternalInput").ap()
    bkT = nc.dram_tensor("bkT", [P, cfg.nt_d], F32, kind="ExternalInput").ap()
    bv_rep = nc.dram_tensor("bv_rep", [P, cfg.dim], F32, kind="ExternalInput").ap()
    srcT_f = nc.dram_tensor("srcT_f", [P, cfg.nt_k], F32, kind="ExternalInput").ap()
    srcT_b = nc.dram_tensor("srcT_b", [P, cfg.nt_k], BF16, kind="ExternalInput").ap()
    tgt_row = nc.dram_tensor("tgt_row", [1, cfg.tq], F32, kind="ExternalInput").ap()
    betaT = nc.dram_tensor(
        "betaT", [cfg.h, cfg.tk, cfg.tq], BF16, kind="ExternalInput"
    ).ap()
    outT = nc.dram_tensor("outT", [cfg.dim, cfg.tq], F32, kind="ExternalOutput").ap()

    with tile.TileContext(nc) as tc, ExitStack() as ctx:
        consts = ctx.enter_context(tc.tile_pool(name="consts", bufs=1))
        proj_out = ctx.enter_context(tc.tile_pool(name="projout", bufs=1))
        dram_s = ctx.enter_context(tc.tile_pool(name="dram_s", bufs=2, space="DRAM"))
        ps_sc = ctx.enter_context(tc.tile_pool(name="ps_sc", bufs=2, space="PSUM"))
        ps_pv = ctx.enter_context(tc.tile_pool(name="ps_pv", bufs=1, space="PSUM"))
        ps_pb = ctx.enter_context(tc.tile_pool(name="ps_pb", bufs=2, space="PSUM"))

        # qpT/kpT: [p, ot, t] bf16
        # vp_m: src-masked, [p, tt, h, 65] (col 64 = src mask); vp_p: plain
        qp_sb = proj_out.tile([P, cfg.nt_d, cfg.tq], BF16, tag="qp")
        kp_sb = proj_out.tile([P, cfg.nt_d, cfg.tk], BF16, tag="kp")
        vp_m = proj_out.tile([P, cfg.nt_k, cfg.h, D + 1], BF16, tag="vpm")
        vp_p = proj_out.tile([P, cfg.nt_k, cfg.h, D], BF16, tag="vpp")

        # ---- projections (weight/input pools scoped: SBUF recycles after) ----
        with tc.tile_pool(name="wpool", bufs=1) as w_pool, tc.tile_pool(
            name="inp", bufs=2
        ) as in_pool:
            w_sb = {}
            CH = max(1, cfg.nt_d // 4)  # dtiles per DMA chunk

            def load_w(name, w, interleave=None):
                t = w_pool.tile(
                    [P, cfg.nt_d, cfg.dim], BF16, tag=f"w_{name}", name=f"w_{name}"
                )
                wr = w.rearrange("(dt p) o -> p dt o", p=P)
                for c in range(cfg.nt_d // CH):
                    cs = slice(c * CH, (c + 1) * CH)
                    nc.sync.dma_start(t[:, cs, :], wr[:, cs, :])
                    if interleave is not None:
                        xtile, xr, tqs = interleave
                        nc.sync.dma_start(xtile[:, cs, :], xr[:, cs, tqs])
                w_sb[name] = t

            # first projection's input is interleaved with its weight chunks
            # so the first matmul group isn't stuck behind the whole queue
            x0 = in_pool.tile([P, cfg.nt_d, cfg.tqb], BF16, tag="xin", name="x0")
            qr0 = qT.rearrange("(dt p) t -> p dt t", p=P)
            load_w("wq", WqT, interleave=(x0, qr0, slice(0, cfg.tqb)))
            # small resident constants (emitted after the first weight chunks
            # so they don't delay the first matmul)
            bq_sb = consts.tile([P, cfg.nt_d], F32, tag="bq")
            nc.sync.dma_start(bq_sb[:], bqT)
            bk_sb = consts.tile([P, cfg.nt_d], F32, tag="bk")
            nc.sync.dma_start(bk_sb[:], bkT)
            bv_sb = consts.tile([P, cfg.dim], F32, tag="bv")
            nc.sync.dma_start(bv_sb[:], bv_rep)
            src_sb = consts.tile([P, cfg.nt_k], F32, tag="src")
            nc.sync.dma_start(src_sb[:], srcT_f)
            srcb_sb = consts.tile([P, cfg.nt_k], BF16, tag="srcb")
            nc.sync.dma_start(srcb_sb[:], srcT_b)
            # tgt row lives at partition 64 (same as the PV denominator row)
            tgt_sb = consts.tile([P, cfg.tq], F32, tag="tgt")
            nc.sync.dma_start(tgt_sb[64:65, :], tgt_row)

            # qpT / kpT  (out tile [o=128, t=tqb])
            for name, wdram, src, dst, bias in (
                ("wq", None, qT, qp_sb, bq_sb),
                ("wk", WkT, kT, kp_sb, bk_sb),
            ):
                if wdram is not None:
                    load_w(name, wdram)
                w = w_sb[name]
                for tb in range(cfg.n_tqb):
                    if name == "wq" and tb == 0:
                        x = x0
                    else:
                        x = in_pool.tile([P, cfg.nt_d, cfg.tqb], BF16, tag="xin")
                        xr = src.rearrange("(dt p) t -> p dt t", p=P)
                        for c in range(cfg.nt_d // CH):
                            cs = slice(c * CH, (c + 1) * CH)
                            nc.sync.dma_start(
                                x[:, cs, :],
                                xr[:, cs, tb * cfg.tqb : (tb + 1) * cfg.tqb],
                            )
                    for ot in range(cfg.nt_d):
                        ps = ps_sc.tile([P, 2, cfg.tqb], F32, tag="ps")
                        for dt in range(cfg.nt_d):
                            nc.tensor.matmul(
                                ps[:, 0, :],
                                w[:, dt, ot * P : (ot + 1) * P],
                                x[:, dt, :],
                                start=(dt == 0),
                                stop=(dt == cfg.nt_d - 1),
                            )
                        nc.vector.tensor_add(
                            dst[:, ot, tb * cfg.tqb : (tb + 1) * cfg.tqb],
                            ps[:, 0, :],
                            bias[:, ot : ot + 1].to_broadcast([P, cfg.tqb]),
                        )

            # vp (out tile [t=128, o=OB]); plain + masked copies
            OB = min(512, cfg.dim)
            n_ob = cfg.dim // OB
            hpb = OB // D  # heads per block
            load_w("wv", WvT)
            wv = w_sb["wv"]
            # v input loaded once (8 chunks), sliced per t-tile below
            xv = in_pool.tile([P, cfg.nt_d, cfg.tk], BF16, tag="xv")
            xvr = vT.rearrange("(dt p) t -> p dt t", p=P)
            for c in range(cfg.nt_d // CH):
                cs = slice(c * CH, (c + 1) * CH)
                nc.sync.dma_start(xv[:, cs, :], xvr[:, cs, :])
            for tt in range(cfg.nt_k):
                x = xv[:, :, tt * P : (tt + 1) * P]
                for ob in range(n_ob):
                    ps = ps_sc.tile([P, 2, cfg.tqb], F32, tag="ps")
                    for dt in range(cfg.nt_d):
                        nc.tensor.matmul(
                            ps[:, 0, :OB],
                            x[:, dt, :],
                            wv[:, dt, ob * OB : (ob + 1) * OB],
                            start=(dt == 0),
                            stop=(dt == cfg.nt_d - 1),
                        )
                    hsl = slice(ob * hpb, (ob + 1) * hpb)
                    nc.vector.tensor_add(
                        vp_p[:, tt, hsl, :],
                        ps[:, 0, :OB].rearrange("p (h d) -> p h d", d=D),
                        bv_sb[:, ob * OB : (ob + 1) * OB].rearrange(
                            "p (h d) -> p h d", d=D
                        ),
                    )
                    nc.vector.tensor_scalar_mul(
                        vp_m[:, tt, hsl, 0:D],
                        vp_p[:, tt, hsl, :],
                        src_sb[:, tt : tt + 1],
                    )

        # src-mask ones column of vp_m (DVE free-dim broadcast, not DMA)
        nc.vector.tensor_copy(
            vp_m[:, :, :, D],
            srcb_sb[:, :, None].to_broadcast([P, cfg.nt_k, cfg.h]),
        )

        # ---- attention: software-pipelined over head pairs ----
        # Per pair: scores+exp run one pair AHEAD of the PV/fixup stage, and
        # their PE instructions are interleaved per k-tile group so the
        # in-order PE queue never waits on the exp (ACT) pipeline.
        e_pool = ctx.enter_context(tc.tile_pool(name="epool", bufs=2))
        b_pool = ctx.enter_context(tc.tile_pool(name="bpool", bufs=4))
        s_pool = ctx.enter_context(tc.tile_pool(name="spool", bufs=2))
        o_pool = ctx.enter_context(tc.tile_pool(name="opool", bufs=3))
        pairs = [
            (tb, j) for tb in range(cfg.n_tqb) for j in range(cfg.h // 2)
        ]
        prev = None  # state of pair idx-1: dict with e_t, bsl, ps_e, ps_b, tqs, j

        def emit_scores_exp(state, kt2):
            """Two heads packed in PE rows 0-63/64-127; one exp per 2 k-tiles."""
            j, tqs = state["j"], state["tqs"]
            for half in range(2):
                r0 = half * 64
                ps = ps_sc.tile([P, 2, cfg.tqb], F32, tag="ps", name="ps")
                for ki in range(2):
                    kt = 2 * kt2 + ki
                    nc.tensor.matmul(
                        ps[:, ki, :],
                        kp_sb[r0 : r0 + 64, j, kt * P : (kt + 1) * P],
                        qp_sb[r0 : r0 + 64, j, tqs],
                        start=True,
                        stop=True,
                    )
                nc.scalar.activation(
                    state["e_t"][half][:, 2 * kt2 : 2 * kt2 + 2, :],
                    ps[:],
                    mybir.ActivationFunctionType.Exp,
                    scale=cfg.scale,
                )

        def emit_pv(state, kt):
            st, sp = kt == 0, kt == cfg.nt_k - 1
            if st:
                state["ps_e"] = [
                    ps_pv.tile([P, cfg.tqb], F32, tag=f"pse{h}", name=f"pse{h}")
                    for h in range(2)
                ]
                # both heads' beta-PV share one bank via column tiling
                # (head0 -> partitions 0-63, head1 -> 64-127): the two
                # independent rhs streams run concurrently on the PE array
                state["ps_b"] = ps_pb.tile([P, cfg.tqb], F32, tag="psb", name="psb")
            for half in range(2):
                hh = 2 * state["j"] + half
                nc.tensor.matmul(
                    state["ps_e"][half][0 : D + 1, :],
                    vp_m[:, kt, hh, :],
                    state["e_t"][half][:, kt, :],
                    start=st,
                    stop=sp,
                )
                nc.tensor.matmul(
                    state["ps_b"][half * D : (half + 1) * D, :],
                    vp_p[:, kt, hh, :],
                    state["bsl"][half][:, kt, :],
                    start=st,
                    stop=sp,
                    tile_position=(0, half * D),
                    skip_group_check=True,
                )

        def emit_fixup(state):
            j, tqs = state["j"], state["tqs"]
            for half in range(2):
                hh = 2 * j + half
                # Drain both PSUM accumulators to SBUF right away (ACT + DVE)
                # so the banks free for the next pair's PV groups; the slow
                # normalization chain below then runs entirely from SBUF.
                oe = o_pool.tile([D + 1, cfg.tqb], F32, tag="oe", name="oe")
                nc.scalar.activation(
                    oe[:],
                    state["ps_e"][half][0 : D + 1, :],
                    mybir.ActivationFunctionType.Copy,
                )
                # s = tgt / denom  (denominator row sits at partition 64)
                srow = s_pool.tile([P, cfg.tqb], F32, tag="srow", name="srow")
                nc.vector.reciprocal(srow[64:65, :], oe[64:65, :])
                nc.vector.tensor_mul(
                    srow[64:65, :], srow[64:65, :], tgt_sb[64:65, tqs]
                )
                s_dram = dram_s.tile([1, cfg.tqb], F32, tag="sdram", name="sdram")
                nc.sync.dma_start(s_dram[:], srow[64:65, :])
                s_rep = s_pool.tile([64, cfg.tqb], F32, tag="srep", name="srep")
                nc.sync.dma_start(s_rep[:], s_dram[:].to_broadcast([64, cfg.tqb]))
                tmp = o_pool.tile([64, cfg.tqb], F32, tag="tmp", name="tmp")
                nc.vector.tensor_mul(tmp[:], oe[0:D, :], s_rep[:])
                # out = tmp + beta-part: bypass write, then DMA-accumulate the
                # beta half (copied to SBUF partition-aligned; DMA can't read
                # PSUM directly)
                ob = o_pool.tile([P, cfg.tqb], F32, tag="ob", name="ob")
                nc.vector.tensor_copy(
                    ob[half * D : (half + 1) * D, :],
                    state["ps_b"][half * D : (half + 1) * D, :],
                )
                nc.sync.dma_start(outT[hh * D : (hh + 1) * D, tqs], tmp[:])
                nc.gpsimd.dma_start(
                    outT[hh * D : (hh + 1) * D, tqs],
                    ob[half * D : (half + 1) * D, :],
                    accum_op=mybir.AluOpType.add,
                )

        for idx, (tb, j) in enumerate(pairs):
            tqs = slice(tb * cfg.tqb, (tb + 1) * cfg.tqb)
            state = {"j": j, "tqs": tqs}
            state["e_t"] = [
                e_pool.tile(
                    [P, cfg.nt_k, cfg.tqb], BF16, tag=f"e{h}", name=f"e{h}"
                )
                for h in range(2)
            ]
            state["bsl"] = []
            for half in range(2):
                hh = 2 * j + half
                bt = b_pool.tile(
                    [P, cfg.nt_k, cfg.tqb], BF16, tag="beta", name=f"beta{half}"
                )
                nc.sync.dma_start(
                    bt[:],
                    betaT[hh].rearrange("(kt p) t -> p kt t", p=P)[:, :, tqs],
                )
                state["bsl"].append(bt)
            for kt2 in range(cfg.nt_k // 2):
                emit_scores_exp(state, kt2)
                if prev is not None:
                    emit_pv(prev, 2 * kt2)
                    emit_pv(prev, 2 * kt2 + 1)
            if prev is not None:
                emit_fixup(prev)
            prev = state
        for kt in range(cfg.nt_k):
            emit_pv(prev, kt)
        emit_fixup(prev)

    nc.compile()
    return nc


def host_prep(cfg: Cfg, q, k, v, beta, src_mask, tgt_mask, Wq, bq, Wk, bk, Wv, bv):
    """Build per-core input maps (host-side sharding + transposition)."""
    WqT = np.ascontiguousarray(Wq.T).astype(NPBF16)
    WkT = np.ascontiguousarray(Wk.T).astype(NPBF16)
    WvT = np.ascontiguousarray(Wv.T).astype(NPBF16)
    bqT = np.ascontiguousarray(bq.reshape(cfg.nt_d, P).T).astype(np.float32)
    bkT = np.ascontiguousarray(bk.reshape(cfg.nt_d, P).T).astype(np.float32)
    bv_rep = np.ascontiguousarray(np.broadcast_to(bv, (P, cfg.dim))).astype(np.float32)
    betaT = np.ascontiguousarray(beta.transpose(0, 2, 1)).astype(NPBF16)

    in_maps = []
    for b in range(q.shape[0]):
        srcT = np.ascontiguousarray(
            src_mask[b].astype(np.float32).reshape(cfg.nt_k, P).T
        )
        in_maps.append(
            {
                "qT": np.ascontiguousarray(q[b].T).astype(NPBF16),
                "kT": np.ascontiguousarray(k[b].T).astype(NPBF16),
                "vT": np.ascontiguousarray(v[b].T).astype(NPBF16),
                "WqT": WqT,
                "WkT": WkT,
                "WvT": WvT,
                "bqT": bqT,
                "bkT": bkT,
                "bv_rep": bv_rep,
                "srcT_f": srcT,
                "srcT_b": srcT.astype(NPBF16),
                "tgt_row": tgt_mask[b].astype(np.float32).reshape(1, cfg.tq),
                "betaT": betaT,
            }
        )
    return in_maps


def host_finish(cfg: Cfg, results, v, tgt_mask, Wv, bv):
    """Assemble full output; patch uniform-softmax rows where tgt_mask=0."""
    nb = v.shape[0]
    out = np.empty((nb, cfg.tq, cfg.dim), np.float32)
    for b in range(nb):
        out[b] = results[b]["outT"].T
        inv = ~tgt_mask[b]
        if inv.any():
            vsum = v[b].sum(axis=0, dtype=np.float64) @ Wv.T.astype(
                np.float64
            ) + cfg.tk * bv.astype(np.float64)
            out[b, inv, :] += (vsum / cfg.tk).astype(np.float32)
    return out


_NC = None


def kernel(q, k, v, beta, src_mask, tgt_mask, Wq, bq, Wk, bk, Wv, bv):
    global _NC
    from concourse.bass_utils import run_bass_kernel_spmd

    q = np.asarray(q, np.float32)
    k = np.asarray(k, np.float32)
    v = np.asarray(v, np.float32)
    beta = np.asarray(beta, np.float32)
    src_mask = np.asarray(src_mask, bool)
    tgt_mask = np.asarray(tgt_mask, bool)
    Wq, bq = np.asarray(Wq, np.float32), np.asarray(bq, np.float32)
    Wk, bk = np.asarray(Wk, np.float32), np.asarray(bk, np.float32)
    Wv, bv = np.asarray(Wv, np.float32), np.asarray(bv, np.float32)

    cfg = Cfg()
    if _NC is None:
        _NC = build_kernel(cfg)
    in_maps = host_prep(cfg, q, k, v, beta, src_mask, tgt_mask, Wq, bq, Wk, bk, Wv, bv)
    res = run_bass_kernel_spmd(_NC, in_maps, list(range(N_CORES)))
    return host_finish(cfg, res.results, v, tgt_mask, Wv, bv)
